# revision 14
# baseline (speedup 1.0000x reference)
"""Trainium2 Bass kernel for a pre-norm transformer block (nn_Block_25752623907165).

Sharding: data-parallel over batch B=8 across the 8 NeuronCores (one batch
element per core, zero collectives).

Per-core dataflow (all activations kept TRANSPOSED: [features, tokens], so
every matmul contracts along the SBUF partition dim with no on-device
transposes; the host pre-transposes x and pre-packs all weights):

  xT[C,N] --LN1--> hT(bf16) --PE--> qT,kT[2C,N] and v[N,C] (augmented w/ ones col)
  per head: scoresT = kT.h' qT (PSUM) --ACT exp--> expT(bf16)
            [v|1]' expT -> PSUM [65,N]: rows 0-63 = o_unnorm', row 64 = denom
  denom rows -> batched DVE reciprocal -> PE K=1 broadcast -> normalize -> oT
  projT = proj_w' oT (+bias, +residual into xT)
  LN2 -> fc1/gelu (bf16) -> fc2 (+bias, +residual) -> yT -> host transposes back

Matmul inputs are bf16 (weights pre-cast on host, activations cast on PSUM
eviction); accumulation, layernorm statistics and the residual stream stay
fp32. Measured end-to-end error of this scheme vs the fp32 reference:
~6e-4 relative (Frobenius).
"""

import numpy as np
import ml_dtypes

EMBED = 1024
HEADS = 16
HEAD_DIM = 64
HIDDEN = 4096
N_TOK = 1024
B = 8
N_CORES = 8
EPS = 1e-5
P = 128
CSUB = EMBED // P          # 8
HSUB = HIDDEN // P         # 32
QT = 2                     # token tiles of 512 (fp32 moving-operand max)
QW = 512

BF16 = ml_dtypes.bfloat16

_CACHE = {}


# ---------------------------------------------------------------------------
# host-side packing helpers
# ---------------------------------------------------------------------------

def _pack_lhsT_chunks(w, n_mtiles):
    """[K, M] fp32 -> [n_mtiles, 128, K//128, 128] bf16, contiguous per-chunk.

    chunk[mt][p, ko, mi] = w[ko*128 + p, mt*128 + mi]  (lhsT tiles for PE).
    """
    K, M = w.shape
    a = w.reshape(K // P, P, n_mtiles, P).transpose(2, 1, 0, 3)
    return np.ascontiguousarray(a.astype(BF16))


def _pack_rhs(w):
    """[K, M] fp32 -> [128, K//128, M] bf16 (moving-operand layout)."""
    K, M = w.shape
    a = w.reshape(K // P, P, M).transpose(1, 0, 2)
    return np.ascontiguousarray(a.astype(BF16))


def _pack_percol(v):
    """[F] fp32 -> [128, F//128] fp32: column m holds features m*128..m*128+127."""
    F = v.shape[0]
    return np.ascontiguousarray(v.reshape(F // P, P).T.astype(np.float32))


def _pack_xT(xb):
    """[N, C] fp32 -> [128, C//128, N] fp32 (transposed, partition-major)."""
    xT = xb.T  # [C, N]
    a = xT.reshape(CSUB, P, N_TOK).transpose(1, 0, 2)
    return np.ascontiguousarray(a.astype(np.float32))


def _unpack_yT(yT):
    """[128, C//128, N] fp32 -> [N, C] fp32."""
    full = yT.transpose(1, 0, 2).reshape(EMBED, N_TOK)  # [C, N]
    return np.ascontiguousarray(full.T)


# ---------------------------------------------------------------------------
# kernel build
# ---------------------------------------------------------------------------

def _build():
    import concourse.bacc as bacc
    import concourse.mybir as mybir
    import concourse.tile as tile
    from contextlib import ExitStack

    dt = mybir.dt
    AF = mybir.ActivationFunctionType
    OP = mybir.AluOpType

    nc = bacc.Bacc("TRN2", target_bir_lowering=False, debug=False)

    f32, bf16 = dt.float32, dt.bfloat16

    def dram(name, shape, d=f32, out=False):
        return nc.dram_tensor(name, list(shape), d,
                              kind="ExternalOutput" if out else "ExternalInput").ap()

    xT_d = dram("xT", [P, CSUB, N_TOK])
    wqk_d = dram("wqk", [16, P, CSUB, P], bf16)      # lhsT chunks, q|k features
    wv_d = dram("wv", [P, CSUB, EMBED], bf16)        # rhs layout
    bqk_d = dram("bqk", [P, 16])
    bv_d = dram("bv", [1, EMBED], bf16)
    wpr_d = dram("wpr", [CSUB, P, CSUB, P], bf16)
    bpr_d = dram("bpr", [P, CSUB])
    wf1_d = dram("wf1", [HSUB, P, CSUB, P], bf16)
    bf1_d = dram("bf1", [P, HSUB])
    wf2_d = dram("wf2", [CSUB, P, HSUB, P], bf16)
    bf2_d = dram("bf2", [P, CSUB])
    g1_d = dram("g1", [P, CSUB])
    b1_d = dram("b1", [P, CSUB])
    g2_d = dram("g2", [P, CSUB])
    b2_d = dram("b2", [P, CSUB])
    yT_d = dram("yT", [P, CSUB, N_TOK], out=True)

    with tile.TileContext(nc) as tc, ExitStack() as ctx:
        const = ctx.enter_context(tc.tile_pool(name="const", bufs=1))
        persist = ctx.enter_context(tc.tile_pool(name="persist", bufs=1))
        smalls = ctx.enter_context(tc.tile_pool(name="smalls", bufs=1))
        tmpf = ctx.enter_context(tc.tile_pool(name="tmpf", bufs=3))
        w8_pool = ctx.enter_context(tc.tile_pool(name="w8", bufs=2))
        exp_pool = ctx.enter_context(tc.tile_pool(name="expp", bufs=3))
        ps = ctx.enter_context(tc.tile_pool(name="ps", bufs=2, space="PSUM"))
        po = ctx.enter_context(tc.tile_pool(name="po", bufs=2, space="PSUM"))

        # ---- constants ---------------------------------------------------
        ones_sq = const.tile([P, P], bf16)      # 1/1024 : layernorm mean lhsT
        nc.vector.memset(ones_sq[:], 1.0 / EMBED)
        ones128 = const.tile([P, P], f32)       # 1.0 : fp32 broadcast lhsT rows
        nc.vector.memset(ones128[:], 1.0)
        ones_bf = const.tile([1, P], bf16)      # 1.0 : vbias broadcast lhsT
        nc.vector.memset(ones_bf[:], 1.0)

        bqk_sb = const.tile([P, 16], f32)
        nc.sync.dma_start(bqk_sb[:], bqk_d[:])
        bv_row = const.tile([1, EMBED], bf16)
        nc.sync.dma_start(bv_row[:], bv_d[:])
        bpr_sb = const.tile([P, CSUB], f32)
        nc.sync.dma_start(bpr_sb[:], bpr_d[:])
        bf1_sb = const.tile([P, HSUB], f32)
        nc.sync.dma_start(bf1_sb[:], bf1_d[:])
        bf2_sb = const.tile([P, CSUB], f32)
        nc.sync.dma_start(bf2_sb[:], bf2_d[:])
        g1_sb = const.tile([P, CSUB], f32)
        nc.sync.dma_start(g1_sb[:], g1_d[:])
        b1_sb = const.tile([P, CSUB], f32)
        nc.sync.dma_start(b1_sb[:], b1_d[:])
        g2_sb = const.tile([P, CSUB], f32)
        nc.sync.dma_start(g2_sb[:], g2_d[:])
        b2_sb = const.tile([P, CSUB], f32)
        nc.sync.dma_start(b2_sb[:], b2_d[:])

        # ---- persistent activations -------------------------------------
        xT = persist.tile([P, CSUB, N_TOK], f32)
        nc.sync.dma_start(xT[:], xT_d[:])
        oT = persist.tile([P, CSUB, N_TOK], bf16)

        # ---- layernorm emitter ------------------------------------------
        def emit_layernorm(x_sb, g_col, b_col, out_sb):
            mu_ps = ps.tile([P, N_TOK], f32, tag="ps")
            sq_ps = ps.tile([P, N_TOK], f32, tag="ps")
            for c in range(CSUB):
                x_b = tmpf.tile([P, N_TOK], bf16, tag="xb")
                nc.vector.tensor_copy(x_b[:], x_sb[:, c, :])
                sq_b = tmpf.tile([P, N_TOK], bf16, tag="sqb")
                nc.scalar.activation(sq_b[:], x_b[:], AF.Square)
                for q in range(QT):
                    sl = slice(q * QW, (q + 1) * QW)
                    nc.tensor.matmul(mu_ps[:, sl], ones_sq[:], x_b[:, sl],
                                     start=(c == 0), stop=(c == CSUB - 1))
                    nc.tensor.matmul(sq_ps[:, sl], ones_sq[:], sq_b[:, sl],
                                     start=(c == 0), stop=(c == CSUB - 1))
            mu_sb = smalls.tile([P, N_TOK], f32, tag="mu")
            a_t = smalls.tile([P, N_TOK], f32, tag="lnA")
            nc.scalar.activation(a_t[:], mu_ps[:], AF.Square)      # mu^2
            nc.scalar.activation(mu_sb[:], mu_ps[:], AF.Copy)
            b_t = smalls.tile([P, N_TOK], f32, tag="lnB")
            nc.vector.tensor_scalar(b_t[:], sq_ps[:], EPS, None, OP.add)
            nc.vector.tensor_tensor(b_t[:], b_t[:], a_t[:], OP.subtract)  # var+eps
            rv = smalls.tile([P, N_TOK], f32, tag="lnA")
            nc.vector.reciprocal(rv[:], b_t[:])
            rstd_sb = smalls.tile([P, N_TOK], f32, tag="rstd")
            nc.scalar.activation(rstd_sb[:], rv[:], AF.Sqrt)
            for c in range(CSUB):
                t = tmpf.tile([P, N_TOK], f32, tag="t4k")
                nc.vector.tensor_tensor(t[:], x_sb[:, c, :], mu_sb[:], OP.subtract)
                nc.vector.tensor_tensor(t[:], t[:], rstd_sb[:], OP.mult)
                nc.vector.tensor_scalar(out_sb[:, c, :], t[:],
                                        g_col[:, c:c + 1], b_col[:, c:c + 1],
                                        OP.mult, OP.add)

        # =================================================================
        # attention superphase
        # =================================================================
        with tc.tile_pool(name="attn_sb", bufs=1) as attn_sb:
            hT = attn_sb.tile([P, CSUB, N_TOK], bf16)
            emit_layernorm(xT, g1_sb, b1_sb, hT)

            v65 = attn_sb.tile([P, CSUB, HEADS, 65], bf16)
            oU = attn_sb.tile([64, 8, N_TOK], bf16)
            # two 8-row denominator tiles: compute ops need partition base 0
            Drows = [attn_sb.tile([8, N_TOK], f32, tag=f"dr{i}", name=f"dr{i}")
                     for i in range(2)]
            invD_all = [attn_sb.tile([8, N_TOK], f32, tag=f"iv{i}", name=f"iv{i}")
                        for i in range(2)]

            # ---- v = hT' wv (normal layout), into v65 = [v | ones] ------
            with tc.tile_pool(name="wv_sb", bufs=1) as wvp:
                wv_sb = wvp.tile([P, CSUB, EMBED], bf16)
                nc.sync.dma_start(wv_sb[:], wv_d[:])

                vb_ps = ps.tile([P, N_TOK], f32, tag="ps")
                for q in range(QT):
                    sl = slice(q * QW, (q + 1) * QW)
                    nc.tensor.matmul(vb_ps[:, sl], ones_bf[:], bv_row[:, sl])
                vb_b = wvp.tile([P, EMBED], f32)
                nc.vector.tensor_copy(vb_b[:], vb_ps[:])

                nc.vector.memset(v65[:, :, :, 64:65], 1.0)
                for mt in range(CSUB):  # token tiles (= key tiles)
                    v_ps = ps.tile([P, N_TOK], f32, tag="ps")
                    for c in range(CSUB):
                        for q in range(QT):
                            sl = slice(q * QW, (q + 1) * QW)
                            nc.tensor.matmul(v_ps[:, sl],
                                             hT[:, c, mt * P:(mt + 1) * P],
                                             wv_sb[:, c, sl],
                                             start=(c == 0), stop=(c == CSUB - 1))
                    nc.vector.tensor_tensor(
                        v65[:, mt, :, 0:64],
                        v_ps[:].rearrange("p (h d) -> p h d", d=64),
                        vb_b[:].rearrange("p (h d) -> p h d", d=64),
                        OP.add)

            # ---- per head pair: qT,kT matmuls then attention ------------
            def normalize_heads(h_lo):
                for h in range(h_lo, h_lo + 8):
                    base = (h % 2) * 64
                    iD64 = tmpf.tile([65, N_TOK], f32, tag="t4k")
                    nc.sync.dma_start(iD64[64:65, :],
                                      invD_all[h // 8][h % 8:h % 8 + 1, :])
                    bc = ps.tile([P, N_TOK], f32, tag="ps")
                    for q in range(QT):
                        sl = slice(q * QW, (q + 1) * QW)
                        nc.tensor.matmul(bc[0:64, sl], ones128[64:65, 0:64],
                                         iD64[64:65, sl])
                    invDb = tmpf.tile([64, N_TOK], f32, tag="t4k")
                    nc.vector.tensor_copy(invDb[:], bc[0:64, :])
                    to = tmpf.tile([64, N_TOK], bf16, tag="t2k")
                    nc.vector.tensor_tensor(to[:], oU[:, h % 8, :], invDb[:],
                                            OP.mult)
                    nc.sync.dma_start(oT[base:base + 64, h // 2, :], to[:])

            for hp in range(8):
                qkp = attn_sb.tile([P, 2, N_TOK], bf16, tag="qkp", bufs=2)
                for i, m in enumerate((hp, 8 + hp)):
                    wch = w8_pool.tile([P, CSUB, P], bf16, tag="w8x128")
                    nc.sync.dma_start(wch[:], wqk_d[m])
                    qk_ps = ps.tile([P, N_TOK], f32, tag="ps")
                    for c in range(CSUB):
                        for q in range(QT):
                            sl = slice(q * QW, (q + 1) * QW)
                            nc.tensor.matmul(qk_ps[:, sl], wch[:, c, :],
                                             hT[:, c, sl],
                                             start=(c == 0), stop=(c == CSUB - 1))
                    nc.vector.tensor_scalar(qkp[:, i, :], qk_ps[:],
                                            bqk_sb[:, m:m + 1], None, OP.add)

                for h in (2 * hp, 2 * hp + 1):
                    base = (h % 2) * 64
                    bs = slice(base, base + 64)
                    o_ps = po.tile([65, N_TOK], f32, tag="po")
                    for k in range(CSUB):
                        s_ps = ps.tile([P, N_TOK], f32, tag="ps")
                        for q in range(QT):
                            sl = slice(q * QW, (q + 1) * QW)
                            nc.tensor.matmul(s_ps[:, sl],
                                             qkp[bs, 1, k * P:(k + 1) * P],
                                             qkp[bs, 0, sl])
                        e_k = exp_pool.tile([P, N_TOK], bf16, tag="exp")
                        nc.scalar.activation(e_k[:], s_ps[:], AF.Exp, scale=0.125)
                        for q in range(QT):
                            sl = slice(q * QW, (q + 1) * QW)
                            nc.tensor.matmul(o_ps[:, sl], v65[:, k, h, :],
                                             e_k[:, sl],
                                             start=(k == 0), stop=(k == CSUB - 1))
                    nc.vector.tensor_copy(oU[:, h % 8, :], o_ps[0:64, :])
                    d64 = tmpf.tile([65, N_TOK], f32, tag="t4k")
                    nc.vector.tensor_copy(d64[64:65, :], o_ps[64:65, :])
                    nc.sync.dma_start(Drows[h // 8][h % 8:h % 8 + 1, :],
                                      d64[64:65, :])

                if hp == 3:
                    nc.vector.reciprocal(invD_all[0][:], Drows[0][:])
                    normalize_heads(0)
            nc.vector.reciprocal(invD_all[1][:], Drows[1][:])
            normalize_heads(8)

        # =================================================================
        # proj + residual (into xT)
        # =================================================================
        for m in range(CSUB):
            wch = w8_pool.tile([P, CSUB, P], bf16, tag="w8x128")
            nc.sync.dma_start(wch[:], wpr_d[m])
            p_ps = ps.tile([P, N_TOK], f32, tag="ps")
            for c in range(CSUB):
                for q in range(QT):
                    sl = slice(q * QW, (q + 1) * QW)
                    nc.tensor.matmul(p_ps[:, sl], wch[:, c, :], oT[:, c, sl],
                                     start=(c == 0), stop=(c == CSUB - 1))
            nc.vector.tensor_tensor(xT[:, m, :], p_ps[:], xT[:, m, :], OP.add)
            nc.vector.tensor_scalar(xT[:, m, :], xT[:, m, :],
                                    bpr_sb[:, m:m + 1], None, OP.add)

        # =================================================================
        # LN2 + MLP (token-split halves) + residual -> yT
        # =================================================================
        with tc.tile_pool(name="mlp_sb", bufs=1) as mlp_sb:
            ln2T = mlp_sb.tile([P, CSUB, N_TOK], bf16)
            emit_layernorm(xT, g2_sb, b2_sb, ln2T)

            with tc.tile_pool(name="w32", bufs=2) as w32_pool:
                for th in range(QT):
                    tl = slice(th * QW, (th + 1) * QW)
                    geluT = mlp_sb.tile([P, HSUB, QW], bf16, tag="gelu", bufs=1)
                    for m in range(HSUB):
                        wch = w8_pool.tile([P, CSUB, P], bf16, tag="w8x128")
                        nc.sync.dma_start(wch[:], wf1_d[m])
                        f_ps = ps.tile([P, QW], f32, tag="ps")
                        for c in range(CSUB):
                            nc.tensor.matmul(f_ps[:], wch[:, c, :], ln2T[:, c, tl],
                                             start=(c == 0), stop=(c == CSUB - 1))
                        nc.scalar.activation(geluT[:, m, :], f_ps[:], AF.Gelu,
                                             bias=bf1_sb[:, m:m + 1])

                    for m2 in range(CSUB):
                        w2ch = w32_pool.tile([P, HSUB, P], bf16, tag="w32x128")
                        nc.sync.dma_start(w2ch[:], wf2_d[m2])
                        y_ps = ps.tile([P, QW], f32, tag="ps")
                        for k in range(HSUB):
                            nc.tensor.matmul(y_ps[:], w2ch[:, k, :],
                                             geluT[:, k, :],
                                             start=(k == 0), stop=(k == HSUB - 1))
                        nc.vector.tensor_tensor(xT[:, m2, tl], y_ps[:],
                                                xT[:, m2, tl], OP.add)
                        nc.vector.tensor_scalar(xT[:, m2, tl], xT[:, m2, tl],
                                                bf2_sb[:, m2:m2 + 1], None, OP.add)
                        nc.sync.dma_start(yT_d[:, m2, tl], xT[:, m2, tl])

    nc.compile()
    return nc


def get_nc():
    if "nc" not in _CACHE:
        _CACHE["nc"] = _build()
    return _CACHE["nc"]


def make_in_maps(x, qkv_w, qkv_b, proj_w, proj_b, fc1_w, fc1_b, fc2_w, fc2_b,
                 ln1_g, ln1_b, ln2_g, ln2_b):
    x = np.asarray(x, np.float32)
    shared = {
        "wqk": _pack_lhsT_chunks(np.asarray(qkv_w, np.float32)[:, :2 * EMBED], 16),
        "wv": _pack_rhs(np.asarray(qkv_w, np.float32)[:, 2 * EMBED:]),
        "bqk": _pack_percol(np.asarray(qkv_b, np.float32)[:2 * EMBED]),
        "bv": np.ascontiguousarray(np.asarray(qkv_b, np.float32)[2 * EMBED:][None, :].astype(BF16)),
        "wpr": _pack_lhsT_chunks(np.asarray(proj_w, np.float32), CSUB),
        "bpr": _pack_percol(np.asarray(proj_b, np.float32)),
        "wf1": _pack_lhsT_chunks(np.asarray(fc1_w, np.float32), HSUB),
        "bf1": _pack_percol(np.asarray(fc1_b, np.float32)),
        "wf2": _pack_lhsT_chunks(np.asarray(fc2_w, np.float32), CSUB),
        "bf2": _pack_percol(np.asarray(fc2_b, np.float32)),
        "g1": _pack_percol(np.asarray(ln1_g, np.float32)),
        "b1": _pack_percol(np.asarray(ln1_b, np.float32)),
        "g2": _pack_percol(np.asarray(ln2_g, np.float32)),
        "b2": _pack_percol(np.asarray(ln2_b, np.float32)),
    }
    return [dict(shared, xT=_pack_xT(x[b])) for b in range(B)]


def kernel(**inputs):
    from concourse.bass_utils import run_bass_kernel_spmd

    nc = get_nc()
    in_maps = make_in_maps(**inputs)
    res = run_bass_kernel_spmd(nc, in_maps, core_ids=list(range(N_CORES)))
    out = np.stack([_unpack_yT(res.results[b]["yT"]) for b in range(B)])
    return out.astype(np.float32)


# revision 17
# speedup vs baseline: 1.0233x; 1.0233x over previous
"""Trainium2 Bass kernel for a pre-norm transformer block (nn_Block_25752623907165).

Sharding: data-parallel over batch B=8 across the 8 NeuronCores (one batch
element per core, zero collectives).

Per-core dataflow (all activations kept TRANSPOSED: [features, tokens], so
every matmul contracts along the SBUF partition dim with no on-device
transposes; the host pre-transposes x and pre-packs all weights):

  xT[C,N] --LN1--> hT(bf16) --PE--> qT,kT[2C,N] and v[N,C] (augmented w/ ones col)
  per head: scoresT = kT.h' qT (PSUM) --ACT exp--> expT(bf16)
            [v|1]' expT -> PSUM [65,N]: rows 0-63 = o_unnorm', row 64 = denom
  denom rows -> batched DVE reciprocal -> PE K=1 broadcast -> normalize -> oT
  projT = proj_w' oT (+bias, +residual into xT)
  LN2 -> fc1/gelu (bf16) -> fc2 (+bias, +residual) -> yT -> host transposes back

Matmul inputs are bf16 (weights pre-cast on host, activations cast on PSUM
eviction); accumulation, layernorm statistics and the residual stream stay
fp32. Measured end-to-end error of this scheme vs the fp32 reference:
~6e-4 relative (Frobenius).
"""

import numpy as np
import ml_dtypes

EMBED = 1024
HEADS = 16
HEAD_DIM = 64
HIDDEN = 4096
N_TOK = 1024
B = 8
N_CORES = 8
EPS = 1e-5
P = 128
CSUB = EMBED // P          # 8
HSUB = HIDDEN // P         # 32
QT = 2                     # token tiles of 512 (fp32 moving-operand max)
QW = 512

BF16 = ml_dtypes.bfloat16

_CACHE = {}


# ---------------------------------------------------------------------------
# host-side packing helpers
# ---------------------------------------------------------------------------

def _pack_lhsT_chunks(w, n_mtiles):
    """[K, M] fp32 -> [n_mtiles, 128, K//128, 128] bf16, contiguous per-chunk.

    chunk[mt][p, ko, mi] = w[ko*128 + p, mt*128 + mi]  (lhsT tiles for PE).
    """
    K, M = w.shape
    a = w.reshape(K // P, P, n_mtiles, P).transpose(2, 1, 0, 3)
    return np.ascontiguousarray(a.astype(BF16))


def _pack_rhs(w):
    """[K, M] fp32 -> [128, K//128, M] bf16 (moving-operand layout)."""
    K, M = w.shape
    a = w.reshape(K // P, P, M).transpose(1, 0, 2)
    return np.ascontiguousarray(a.astype(BF16))


def _pack_percol(v):
    """[F] fp32 -> [128, F//128] fp32: column m holds features m*128..m*128+127."""
    F = v.shape[0]
    return np.ascontiguousarray(v.reshape(F // P, P).T.astype(np.float32))


def _pack_xT(xb):
    """[N, C] fp32 -> [128, C//128, N] fp32 (transposed, partition-major)."""
    xT = xb.T  # [C, N]
    a = xT.reshape(CSUB, P, N_TOK).transpose(1, 0, 2)
    return np.ascontiguousarray(a.astype(np.float32))


def _unpack_yT(yT):
    """[128, C//128, N] fp32 -> [N, C] fp32."""
    full = yT.transpose(1, 0, 2).reshape(EMBED, N_TOK)  # [C, N]
    return np.ascontiguousarray(full.T)


# ---------------------------------------------------------------------------
# kernel build
# ---------------------------------------------------------------------------

def _build():
    import concourse.bacc as bacc
    import concourse.mybir as mybir
    import concourse.tile as tile
    from contextlib import ExitStack

    dt = mybir.dt
    AF = mybir.ActivationFunctionType
    OP = mybir.AluOpType

    nc = bacc.Bacc("TRN2", target_bir_lowering=False, debug=False)

    f32, bf16 = dt.float32, dt.bfloat16

    def dram(name, shape, d=f32, out=False):
        return nc.dram_tensor(name, list(shape), d,
                              kind="ExternalOutput" if out else "ExternalInput").ap()

    xT_d = dram("xT", [P, CSUB, N_TOK])
    wqk_d = dram("wqk", [16, P, CSUB, P], bf16)      # lhsT chunks, q|k features
    wv_d = dram("wv", [P, CSUB, EMBED], bf16)        # rhs layout
    bqk_d = dram("bqk", [P, 16])
    bv_d = dram("bv", [1, EMBED], bf16)
    wpr_d = dram("wpr", [CSUB, P, CSUB, P], bf16)
    bpr_d = dram("bpr", [P, CSUB])
    wf1_d = dram("wf1", [HSUB, P, CSUB, P], bf16)
    bf1_d = dram("bf1", [P, HSUB])
    wf2_d = dram("wf2", [CSUB, P, HSUB, P], bf16)
    bf2_d = dram("bf2", [P, CSUB])
    g1_d = dram("g1", [P, CSUB])
    b1_d = dram("b1", [P, CSUB])
    g2_d = dram("g2", [P, CSUB])
    b2_d = dram("b2", [P, CSUB])
    yT_d = dram("yT", [P, CSUB, N_TOK], out=True)

    with tile.TileContext(nc) as tc, ExitStack() as ctx:
        const = ctx.enter_context(tc.tile_pool(name="const", bufs=1))
        persist = ctx.enter_context(tc.tile_pool(name="persist", bufs=1))
        smalls = ctx.enter_context(tc.tile_pool(name="smalls", bufs=1))
        tmpf = ctx.enter_context(tc.tile_pool(name="tmpf", bufs=3))
        w8_pool = ctx.enter_context(tc.tile_pool(name="w8", bufs=2))
        exp_pool = ctx.enter_context(tc.tile_pool(name="expp", bufs=3))
        ps = ctx.enter_context(tc.tile_pool(name="ps", bufs=2, space="PSUM"))
        po = ctx.enter_context(tc.tile_pool(name="po", bufs=2, space="PSUM"))

        # ---- constants ---------------------------------------------------
        ones_sq = const.tile([P, P], bf16)      # 1/1024 : layernorm mean lhsT
        nc.vector.memset(ones_sq[:], 1.0 / EMBED)
        ones_bf = const.tile([P, P], bf16)      # 1.0 : broadcast lhsT rows
        nc.vector.memset(ones_bf[:], 1.0)

        bqk_sb = const.tile([P, 16], f32)
        nc.sync.dma_start(bqk_sb[:], bqk_d[:])
        bv_row = const.tile([1, EMBED], bf16)
        nc.sync.dma_start(bv_row[:], bv_d[:])
        bpr_sb = const.tile([P, CSUB], f32)
        nc.sync.dma_start(bpr_sb[:], bpr_d[:])
        bf1_sb = const.tile([P, HSUB], f32)
        nc.sync.dma_start(bf1_sb[:], bf1_d[:])
        bf2_sb = const.tile([P, CSUB], f32)
        nc.sync.dma_start(bf2_sb[:], bf2_d[:])
        g1_sb = const.tile([P, CSUB], f32)
        nc.sync.dma_start(g1_sb[:], g1_d[:])
        b1_sb = const.tile([P, CSUB], f32)
        nc.sync.dma_start(b1_sb[:], b1_d[:])
        g2_sb = const.tile([P, CSUB], f32)
        nc.sync.dma_start(g2_sb[:], g2_d[:])
        b2_sb = const.tile([P, CSUB], f32)
        nc.sync.dma_start(b2_sb[:], b2_d[:])

        # ---- persistent activations -------------------------------------
        xT = persist.tile([P, CSUB, N_TOK], f32)
        nc.sync.dma_start(xT[:], xT_d[:])
        oT = persist.tile([P, CSUB, N_TOK], bf16)

        # ---- layernorm emitter ------------------------------------------
        def emit_layernorm(x_sb, g_col, b_col, out_sb):
            mu_ps = ps.tile([P, N_TOK], f32, tag="ps")
            sq_ps = ps.tile([P, N_TOK], f32, tag="ps")
            for c in range(CSUB):
                x_b = tmpf.tile([P, N_TOK], bf16, tag="xb", bufs=2)
                nc.vector.tensor_copy(x_b[:], x_sb[:, c, :])
                sq_b = tmpf.tile([P, N_TOK], bf16, tag="sqb", bufs=2)
                nc.scalar.activation(sq_b[:], x_b[:], AF.Square)
                for q in range(QT):
                    sl = slice(q * QW, (q + 1) * QW)
                    nc.tensor.matmul(mu_ps[:, sl], ones_sq[:], x_b[:, sl],
                                     start=(c == 0), stop=(c == CSUB - 1))
                    nc.tensor.matmul(sq_ps[:, sl], ones_sq[:], sq_b[:, sl],
                                     start=(c == 0), stop=(c == CSUB - 1))
            mu_sb = smalls.tile([P, N_TOK], f32, tag="mu")
            a_t = smalls.tile([P, N_TOK], f32, tag="lnA")
            nc.scalar.activation(a_t[:], mu_ps[:], AF.Square)      # mu^2
            nc.scalar.activation(mu_sb[:], mu_ps[:], AF.Copy)
            b_t = smalls.tile([P, N_TOK], f32, tag="lnB")
            nc.vector.tensor_scalar(b_t[:], sq_ps[:], EPS, None, OP.add)
            nc.vector.tensor_tensor(b_t[:], b_t[:], a_t[:], OP.subtract)  # var+eps
            rv = smalls.tile([P, N_TOK], f32, tag="lnA")
            nc.vector.reciprocal_approx_fast(rv[:], b_t[:])
            rstd_sb = smalls.tile([P, N_TOK], f32, tag="rstd")
            nc.scalar.activation(rstd_sb[:], rv[:], AF.Sqrt)
            for c in range(CSUB):
                t = tmpf.tile([P, N_TOK], f32, tag="t4k")
                nc.vector.tensor_tensor(t[:], x_sb[:, c, :], mu_sb[:], OP.subtract)
                nc.vector.tensor_tensor(t[:], t[:], rstd_sb[:], OP.mult)
                nc.vector.tensor_scalar(out_sb[:, c, :], t[:],
                                        g_col[:, c:c + 1], b_col[:, c:c + 1],
                                        OP.mult, OP.add)

        # =================================================================
        # attention superphase
        # =================================================================
        with tc.tile_pool(name="attn_sb", bufs=1) as attn_sb:
            hT = attn_sb.tile([P, CSUB, N_TOK], bf16)
            emit_layernorm(xT, g1_sb, b1_sb, hT)

            v65 = attn_sb.tile([P, CSUB, HEADS, 65], bf16)
            oU = attn_sb.tile([64, 8, N_TOK], bf16)
            # two 8-row denominator tiles: compute ops need partition base 0
            Drows = [attn_sb.tile([8, N_TOK], f32, tag=f"dr{i}", name=f"dr{i}")
                     for i in range(2)]
            invD_all = [attn_sb.tile([8, N_TOK], f32, tag=f"iv{i}", name=f"iv{i}")
                        for i in range(2)]
            invD_bf = [attn_sb.tile([8, N_TOK], bf16, tag=f"ivb{i}", name=f"ivb{i}")
                       for i in range(2)]

            # ---- v = hT' wv (normal layout), into v65 = [v | ones] ------
            with tc.tile_pool(name="wv_sb", bufs=1) as wvp:
                wv_sb = wvp.tile([P, CSUB, EMBED], bf16)
                nc.sync.dma_start(wv_sb[:], wv_d[:])

                vb_ps = ps.tile([P, N_TOK], f32, tag="ps")
                for q in range(QT):
                    sl = slice(q * QW, (q + 1) * QW)
                    nc.tensor.matmul(vb_ps[:, sl], ones_bf[0:1, :], bv_row[:, sl])
                vb_b = wvp.tile([P, EMBED], f32)
                nc.vector.tensor_copy(vb_b[:], vb_ps[:])

                nc.vector.memset(v65[:, :, :, 64:65], 1.0)
                for mt in range(CSUB):  # token tiles (= key tiles)
                    v_ps = ps.tile([P, N_TOK], f32, tag="ps")
                    for c in range(CSUB):
                        for q in range(QT):
                            sl = slice(q * QW, (q + 1) * QW)
                            nc.tensor.matmul(v_ps[:, sl],
                                             hT[:, c, mt * P:(mt + 1) * P],
                                             wv_sb[:, c, sl],
                                             start=(c == 0), stop=(c == CSUB - 1))
                    nc.vector.tensor_tensor(
                        v65[:, mt, :, 0:64],
                        v_ps[:].rearrange("p (h d) -> p h d", d=64),
                        vb_b[:].rearrange("p (h d) -> p h d", d=64),
                        OP.add)

            # ---- per head pair: qT,kT matmuls then attention ------------
            def normalize_heads(h_lo):
                for h in range(h_lo, h_lo + 8):
                    base = (h % 2) * 64
                    iD64 = tmpf.tile([65, N_TOK], bf16, tag="id64b", bufs=2)
                    nc.sync.dma_start(iD64[64:65, :],
                                      invD_bf[h // 8][h % 8:h % 8 + 1, :])
                    bc = ps.tile([P, N_TOK], f32, tag="ps")
                    for q in range(QT):
                        sl = slice(q * QW, (q + 1) * QW)
                        nc.tensor.matmul(bc[0:64, sl], ones_bf[64:65, 0:64],
                                         iD64[64:65, sl])
                    invDb = tmpf.tile([64, N_TOK], f32, tag="t4k")
                    nc.vector.tensor_copy(invDb[:], bc[0:64, :])
                    to = tmpf.tile([64, N_TOK], bf16, tag="t2k")
                    nc.vector.tensor_tensor(to[:], oU[:, h % 8, :], invDb[:],
                                            OP.mult)
                    nc.sync.dma_start(oT[base:base + 64, h // 2, :], to[:])

            for hp in range(8):
                qkp = attn_sb.tile([P, 2, N_TOK], bf16, tag="qkp", bufs=2)
                for i, m in enumerate((hp, 8 + hp)):
                    wch = w8_pool.tile([P, CSUB, P], bf16, tag="w8x128")
                    nc.sync.dma_start(wch[:], wqk_d[m])
                    qk_ps = ps.tile([P, N_TOK], f32, tag="ps")
                    for c in range(CSUB):
                        for q in range(QT):
                            sl = slice(q * QW, (q + 1) * QW)
                            nc.tensor.matmul(qk_ps[:, sl], wch[:, c, :],
                                             hT[:, c, sl],
                                             start=(c == 0), stop=(c == CSUB - 1))
                    nc.vector.tensor_scalar(qkp[:, i, :], qk_ps[:],
                                            bqk_sb[:, m:m + 1], None, OP.add)

                if hp == 4:
                    # heads 0-7 normalize here: their oU slots are reread
                    # before heads 8-9 (emitted below) recycle them, and the
                    # PE chews hp4's qk matmuls while the reciprocal lands.
                    normalize_heads(0)

                for h in (2 * hp, 2 * hp + 1):
                    base = (h % 2) * 64
                    bs = slice(base, base + 64)
                    o_ps = po.tile([65, N_TOK], f32, tag="po")
                    for k in range(CSUB):
                        s_ps = ps.tile([P, N_TOK], f32, tag="ps")
                        for q in range(QT):
                            sl = slice(q * QW, (q + 1) * QW)
                            nc.tensor.matmul(s_ps[:, sl],
                                             qkp[bs, 1, k * P:(k + 1) * P],
                                             qkp[bs, 0, sl])
                        e_k = exp_pool.tile([P, N_TOK], bf16, tag="exp")
                        nc.scalar.activation(e_k[:], s_ps[:], AF.Exp, scale=0.125)
                        for q in range(QT):
                            sl = slice(q * QW, (q + 1) * QW)
                            nc.tensor.matmul(o_ps[:, sl], v65[:, k, h, :],
                                             e_k[:, sl],
                                             start=(k == 0), stop=(k == CSUB - 1))
                    nc.vector.tensor_copy(oU[:, h % 8, :], o_ps[0:64, :])
                    d64 = tmpf.tile([65, N_TOK], f32, tag="t4k")
                    nc.vector.tensor_copy(d64[64:65, :], o_ps[64:65, :])
                    nc.sync.dma_start(Drows[h // 8][h % 8:h % 8 + 1, :],
                                      d64[64:65, :])

                if hp == 3:
                    nc.vector.reciprocal_approx_fast(invD_all[0][:], Drows[0][:])
                    nc.vector.tensor_copy(invD_bf[0][:], invD_all[0][:])
            nc.vector.reciprocal_approx_fast(invD_all[1][:], Drows[1][:])
            nc.vector.tensor_copy(invD_bf[1][:], invD_all[1][:])
            normalize_heads(8)

        # =================================================================
        # proj + residual (into xT)
        # =================================================================
        for m in range(CSUB):
            wch = w8_pool.tile([P, CSUB, P], bf16, tag="w8x128")
            nc.sync.dma_start(wch[:], wpr_d[m])
            p_ps = ps.tile([P, N_TOK], f32, tag="ps")
            for c in range(CSUB):
                for q in range(QT):
                    sl = slice(q * QW, (q + 1) * QW)
                    nc.tensor.matmul(p_ps[:, sl], wch[:, c, :], oT[:, c, sl],
                                     start=(c == 0), stop=(c == CSUB - 1))
            nc.vector.tensor_tensor(xT[:, m, :], p_ps[:], xT[:, m, :], OP.add)
            nc.vector.tensor_scalar(xT[:, m, :], xT[:, m, :],
                                    bpr_sb[:, m:m + 1], None, OP.add)

        # =================================================================
        # LN2 + MLP (token-split halves) + residual -> yT
        # =================================================================
        with tc.tile_pool(name="mlp_sb", bufs=1) as mlp_sb:
            ln2T = mlp_sb.tile([P, CSUB, N_TOK], bf16)
            emit_layernorm(xT, g2_sb, b2_sb, ln2T)

            with tc.tile_pool(name="w32", bufs=2) as w32_pool:
                for th in range(QT):
                    tl = slice(th * QW, (th + 1) * QW)
                    geluT = mlp_sb.tile([P, HSUB, QW], bf16, tag="gelu", bufs=1)
                    for m in range(HSUB):
                        wch = w8_pool.tile([P, CSUB, P], bf16, tag="w8x128")
                        nc.sync.dma_start(wch[:], wf1_d[m])
                        f_ps = ps.tile([P, QW], f32, tag="ps")
                        for c in range(CSUB):
                            nc.tensor.matmul(f_ps[:], wch[:, c, :], ln2T[:, c, tl],
                                             start=(c == 0), stop=(c == CSUB - 1))
                        nc.scalar.activation(geluT[:, m, :], f_ps[:], AF.Gelu,
                                             bias=bf1_sb[:, m:m + 1])

                    for m2 in range(CSUB):
                        w2ch = w32_pool.tile([P, HSUB, P], bf16, tag="w32x128")
                        nc.sync.dma_start(w2ch[:], wf2_d[m2])
                        y_ps = ps.tile([P, QW], f32, tag="ps")
                        for k in range(HSUB):
                            nc.tensor.matmul(y_ps[:], w2ch[:, k, :],
                                             geluT[:, k, :],
                                             start=(k == 0), stop=(k == HSUB - 1))
                        nc.vector.tensor_tensor(xT[:, m2, tl], y_ps[:],
                                                xT[:, m2, tl], OP.add)
                        nc.vector.tensor_scalar(xT[:, m2, tl], xT[:, m2, tl],
                                                bf2_sb[:, m2:m2 + 1], None, OP.add)
                        nc.sync.dma_start(yT_d[:, m2, tl], xT[:, m2, tl])

    nc.compile()
    return nc


def get_nc():
    if "nc" not in _CACHE:
        _CACHE["nc"] = _build()
    return _CACHE["nc"]


def make_in_maps(x, qkv_w, qkv_b, proj_w, proj_b, fc1_w, fc1_b, fc2_w, fc2_b,
                 ln1_g, ln1_b, ln2_g, ln2_b):
    x = np.asarray(x, np.float32)
    shared = {
        "wqk": _pack_lhsT_chunks(np.asarray(qkv_w, np.float32)[:, :2 * EMBED], 16),
        "wv": _pack_rhs(np.asarray(qkv_w, np.float32)[:, 2 * EMBED:]),
        "bqk": _pack_percol(np.asarray(qkv_b, np.float32)[:2 * EMBED]),
        "bv": np.ascontiguousarray(np.asarray(qkv_b, np.float32)[2 * EMBED:][None, :].astype(BF16)),
        "wpr": _pack_lhsT_chunks(np.asarray(proj_w, np.float32), CSUB),
        "bpr": _pack_percol(np.asarray(proj_b, np.float32)),
        "wf1": _pack_lhsT_chunks(np.asarray(fc1_w, np.float32), HSUB),
        "bf1": _pack_percol(np.asarray(fc1_b, np.float32)),
        "wf2": _pack_lhsT_chunks(np.asarray(fc2_w, np.float32), CSUB),
        "bf2": _pack_percol(np.asarray(fc2_b, np.float32)),
        "g1": _pack_percol(np.asarray(ln1_g, np.float32)),
        "b1": _pack_percol(np.asarray(ln1_b, np.float32)),
        "g2": _pack_percol(np.asarray(ln2_g, np.float32)),
        "b2": _pack_percol(np.asarray(ln2_b, np.float32)),
    }
    return [dict(shared, xT=_pack_xT(x[b])) for b in range(B)]


def kernel(**inputs):
    from concourse.bass_utils import run_bass_kernel_spmd

    nc = get_nc()
    in_maps = make_in_maps(**inputs)
    res = run_bass_kernel_spmd(nc, in_maps, core_ids=list(range(N_CORES)))
    out = np.stack([_unpack_yT(res.results[b]["yT"]) for b in range(B)])
    return out.astype(np.float32)
